# revision 55
# baseline (speedup 1.0000x reference)
"""GRN-biased attention (nn_GRNAttentionBias) Trainium2 Bass kernel.

Problem (hardcoded shapes): BH = H = 8 heads, S = 2048, D = 64.
  scores = Q @ K^T / sqrt(D) + head_scale[h] * grn_bias        (per head)
  attn   = softmax(scores, axis=-1)
  out    = attn @ V
  returns (out (1,H,S,D), attn (1,H,S,S))

Sharding: head-parallel — one head per NeuronCore (8 cores).

Kernel design (measured ~89 us/core on TRN2, vs ~99 us of raw HBM traffic
at per-core bandwidth for the fp32 version; bf16 streams bring the traffic
to ~19 MB/core):
  - Everything on-chip is computed TRANSPOSED (scores^T[k,q] instead of
    scores[q,k]) so the attn @ V contraction (over k) has k on partitions
    for both operands (V is naturally [k, d]; e^T is [k, q]) — no on-chip
    transpose of the 4.2M-element attention matrix is ever needed.
  - Row-sums of exp come free by augmenting V with a ones column (the AV
    matmul's 65th output row is Z).
  - grn_bias^T (host-transposed during sharding) is injected into the score
    PSUM by one extra accumulating matmul per tile: stationary = hs*I,
    moving = grn^T tile, computing += hs * grn^T.
  - softmax skips the max-subtraction: scores are bounded (|s| <~ 8 for
    randn Q,K with d=64 plus a [0,1] bias), exp stays in fp32 range.
  - 1/Z is broadcast across partitions on the otherwise-idle GPSIMD
    engine (partition_broadcast); normalize runs on DVE in the bf16 2x
    packed mode.
  - The q axis is processed in chunks [512,512,512,256,256] with each
    chunk's epilogue software-pipelined into the next chunk's k-loop; the
    narrower final chunks shorten the unavoidable post-Z tail.
  - grn^T stays SBUF-resident (bf16, 64KB/partition); its column loads are
    interleaved one chunk ahead so input DMA overlaps output bursts.
  - A few dummy matmuls at trace start warm the PE HAM clock gate while
    the first input DMAs stream in.
  - bf16 is used for the two dominant HBM streams (grn^T read, attn write)
    and the QK / AV matmul operands; PSUM accumulation and the final
    normalization stay fp32. Measured rel-l2 error vs the fp32 reference:
    ~4e-3 on both outputs (quantization-dominated, seed-stable).
The big outputs are written to HBM transposed ([k,q] / [d,q], fully
contiguous DMA) and transposed back on the host during unshard.

Host-side prep (part of sharding): per-head slices Q^T/8 (duplicated onto
both partition halves), K^T (even/odd k-tiles packed onto partition
halves), [V|1], hs*I, and one shared grn^T are computed in numpy.
"""

import os

import numpy as np

H = 8
S = 2048
D = 64
QC = 512  # q-chunk (PSUM bank = 512 fp32)
KT = 128  # k-tile (partition dim)
NQ = S // QC
NK = S // KT
N_CORES = 8

TRACE = bool(int(os.environ.get("KERNEL_TRACE", "0")))
# Halve the two dominant HBM streams (grn^T read, attn write) with bf16.
BF16_GRN = bool(int(os.environ.get("KERNEL_BF16_GRN", "1")))
BF16_ATTN = bool(int(os.environ.get("KERNEL_BF16_ATTN", "1")))
BF16_AV = bool(int(os.environ.get("KERNEL_BF16_AV", "1")))
BF16_QK = bool(int(os.environ.get("KERNEL_BF16_QK", "1")))
# k-tiles per chunk whose grn-bias add runs on DVE instead of a PE matmul
# (PE is the bottleneck engine; DVE has headroom)
DVE_GRN = int(os.environ.get("KERNEL_DVE_GRN", "4"))

_built = None
LAST_RESULT = None  # BassKernelResults from the most recent run (for test.py)


def _build():
    import concourse.mybir as mybir
    from contextlib import ExitStack
    from concourse import bacc
    from concourse.tile import TileContext

    fp32 = mybir.dt.float32
    Exp = mybir.ActivationFunctionType.Exp

    f32r = mybir.dt.float32r  # PE "rapid" fp32: 4x matmul throughput
    bf16 = mybir.dt.bfloat16
    grn_dt = bf16 if BF16_GRN else f32r
    attn_dt = bf16 if BF16_ATTN else fp32
    av_dt = bf16 if BF16_AV else f32r  # exp tiles + V for the AV matmul
    qk_dt = bf16 if BF16_QK else f32r  # Q^T/K^T operands (bf16 enables FWL)

    nc = bacc.Bacc("TRN2", target_bir_lowering=False)

    qT8 = nc.dram_tensor("qT8", [2 * D, S], qk_dt, kind="ExternalInput")
    kT = nc.dram_tensor("kT", [2 * D, S // 2], qk_dt, kind="ExternalInput")
    vaug = nc.dram_tensor("vaug", [S, D + 1], av_dt, kind="ExternalInput")
    grnT = nc.dram_tensor("grnT", [S, S], grn_dt, kind="ExternalInput")
    hsI = nc.dram_tensor("hsI", [KT, KT], grn_dt, kind="ExternalInput")
    attnT = nc.dram_tensor("attnT", [S, S], attn_dt, kind="ExternalOutput")
    outT = nc.dram_tensor("outT", [D, S], fp32, kind="ExternalOutput")

    with TileContext(nc) as tc, ExitStack() as ctx:
        singles = ctx.enter_context(tc.tile_pool(name="singles", bufs=1))
        grn_pool = ctx.enter_context(tc.tile_pool(name="grn", bufs=20))
        e_pool = ctx.enter_context(tc.tile_pool(name="e", bufs=40))
        a_pool = ctx.enter_context(tc.tile_pool(name="a", bufs=6))
        bc_pool = ctx.enter_context(tc.tile_pool(name="bc", bufs=2))
        sm_pool = ctx.enter_context(tc.tile_pool(name="sm", bufs=4))
        sadd_pool = ctx.enter_context(tc.tile_pool(name="sadd", bufs=3))
        ps_pool = ctx.enter_context(tc.tile_pool(name="ps", bufs=5, space="PSUM"))
        po_pool = ctx.enter_context(tc.tile_pool(name="po", bufs=2, space="PSUM"))
        pb_pool = ctx.enter_context(tc.tile_pool(name="pb", bufs=1, space="PSUM"))

        # chunks of the q axis: big chunks first, small last ones so the
        # unavoidable epilogue tail (normalize + attn writes after the final
        # Z is known) is short
        chunks = [(0, 512), (512, 512), (1024, 512), (1536, 256), (1792, 256)]
        # k-tiles whose grn add runs on DVE (spread across the loop)
        dve_grn_set = {3, 7, 11, 14, 1, 5, 9, 13}
        dve_grn_set = (
            set(sorted(dve_grn_set)[:DVE_GRN]) if (DVE_GRN and BF16_GRN) else set()
        )

        # startup-ordered input DMAs: everything chunk 0's k-loop touches
        # first, at fine granularity, so PE starts within a few us
        # row-packed for PE tile concurrency: qT_s = Q^T/8 duplicated onto
        # both partition halves; kT_s = K^T with even k-tiles on partitions
        # 0-63 and odd k-tiles on 64-127 (pair t occupies cols t*128..)
        qT_s = singles.tile([2 * D, S], qk_dt)
        kT_s = singles.tile([2 * D, S // 2], qk_dt)
        hsI_s = singles.tile([KT, KT], grn_dt)
        v_s = singles.tile([128, NK, D + 1], av_dt)  # V' = [V | 1]
        if BF16_GRN:
            g_all = singles.tile([128, NK, S], grn_dt, tag="g_all")
        else:
            g_all = None

        nc.sync.dma_start(out=hsI_s, in_=hsI[:, :])
        q0_0, qw_0 = chunks[0]
        nc.sync.dma_start(out=kT_s[:, 0:256], in_=kT[:, 0:256])
        nc.sync.dma_start(out=qT_s[:, q0_0 : qw_0], in_=qT8[:, q0_0 : qw_0])
        nc.sync.dma_start(out=kT_s[:, 256 : S // 2], in_=kT[:, 256 : S // 2])
        if g_all is not None:
            # first chunk's grn column in 4 sub-DMAs (k-tiles 0-3, 4-7, ...)
            for s in range(4):
                nc.sync.dma_start(
                    out=g_all[:, 4 * s : 4 * s + 4, q0_0 : qw_0],
                    in_=grnT[4 * s * KT : (4 * s + 4) * KT, q0_0 : qw_0].rearrange(
                        "(t p) q -> p t q", p=128
                    ),
                )
        nc.sync.dma_start(out=v_s, in_=vaug[:, :].rearrange("(t p) c -> p t c", p=128))
        for q0, qw in chunks[1:]:
            nc.sync.dma_start(out=qT_s[:, q0 : q0 + qw], in_=qT8[:, q0 : q0 + qw])
        ones1 = singles.tile([1, KT], fp32)
        nc.vector.memset(ones1, 1.0)
        # warm the PE (HAM clock gate) while the first input DMAs stream in:
        # dummy matmuls with no DMA dependencies (full-K shape: K=1 matmuls
        # are pathologically slow)
        warm_w = sm_pool.tile([KT, KT], fp32, tag="warm")
        nc.vector.memset(warm_w, 0.0)
        pwarm = pb_pool.tile([KT, QC], fp32, tag="pb")
        for _ in range(4):
            nc.tensor.matmul(
                pwarm[:, :KT], warm_w, warm_w, start=True, stop=True
            )

        def load_grn_col(q0, qw):
            if g_all is not None:
                nc.sync.dma_start(
                    out=g_all[:, :, q0 : q0 + qw],
                    in_=grnT[:, q0 : q0 + qw].rearrange("(t p) q -> p t q", p=128),
                )

        # chunk 1's grn column up front; later columns are queued one chunk
        # ahead inside the loop so input DMAs interleave with output bursts
        if len(chunks) > 1:
            load_grn_col(*chunks[1])

        def make_epilogue(q0, qw, po, es, wg=4):
            # everything that needs the finished po of chunk (q0, qw);
            # wg = k-tiles per attn write DMA (smaller on the final chunk so
            # the tail drains sooner)
            def epilogue():
                r = sm_pool.tile([1, QC], fp32, tag="r")
                nc.vector.reciprocal(r[:, :qw], po[D : D + 1, :qw])
                bc = bc_pool.tile([KT, QC], av_dt)
                # GPSIMD partition-broadcast: fast (~0.5us) and off the busy PE
                rb = sm_pool.tile([1, QC], av_dt, tag="rb")
                nc.vector.tensor_copy(rb[:, :qw], r[:, :qw])
                nc.gpsimd.partition_broadcast(bc[:, :qw], rb[:, :qw])

                for t in range(NK // wg):
                    a = a_pool.tile([KT, 4, QC], attn_dt, tag="a")
                    for j in range(wg):
                        nc.vector.tensor_mul(
                            a[:, j, :qw], es[wg * t + j][:, :qw], bc[:, :qw]
                        )
                    nc.sync.dma_start(
                        out=attnT[
                            wg * t * KT : (wg * t + wg) * KT, q0 : q0 + qw
                        ].rearrange("(j p) q -> p j q", p=128),
                        in_=a[:, :wg, :qw],
                    )

                ot = sm_pool.tile([D, QC], fp32, tag="ot")
                nc.vector.tensor_mul(ot[:, :qw], po[0:D, :qw], bc[0:D, :qw])
                nc.sync.dma_start(out=outT[:, q0 : q0 + qw], in_=ot[:, :qw])

            return epilogue

        pending_epilogue = None
        for ci, (q0, qw) in enumerate(chunks):
            if ci + 2 < len(chunks):
                load_grn_col(*chunks[ci + 2])
            po = po_pool.tile([D + 1, QC], fp32)
            es = []
            pending = []
            for pt in range(NK // 2):
                pss = []
                for b in range(2):
                    kt = 2 * pt + b
                    # scores^T[k, q] = sum_d K^T[d, k] * (Q^T/8)[d, q]
                    # even/odd k-tiles on partition halves -> the two QK
                    # matmuls run concurrently in the PE array (row groups)
                    ps_full = ps_pool.tile([KT, QC], fp32, tag="ps")
                    ps = ps_full[:, :qw]
                    on_dve = kt % NK in dve_grn_set
                    nc.tensor.matmul(
                        ps,
                        kT_s[b * D : (b + 1) * D, pt * KT : (pt + 1) * KT],
                        qT_s[b * D : (b + 1) * D, q0 : q0 + qw],
                        start=True,
                        stop=on_dve,
                        skip_group_check=True,
                    )
                    pss.append((kt, ps))
                for kt, ps in pss:
                    # += hs*grn^T: PE identity-matmul for most tiles; plain
                    # DVE add (into SBUF) for dve_grn_set to offload the
                    # bottleneck engine
                    if g_all is not None:
                        g = g_all[:, kt, q0 : q0 + qw]
                    else:
                        g_t = grn_pool.tile([KT, QC], grn_dt, tag="grn")
                        g = g_t[:, :qw]
                        nc.sync.dma_start(
                            out=g,
                            in_=grnT[kt * KT : (kt + 1) * KT, q0 : q0 + qw],
                        )
                    if kt % NK not in dve_grn_set:
                        nc.tensor.matmul(
                            ps, hsI_s, g, start=False, stop=True,
                            skip_group_check=True,
                        )
                # deferred AV matmuls so exp (ACT) overlaps PE
                for i, e_prev in pending:
                    nc.tensor.matmul(
                        po[:, :qw],
                        v_s[:, i, :],
                        e_prev[:, :qw],
                        start=(i == 0),
                        stop=False,
                        skip_group_check=True,
                    )
                pending = []
                for kt, ps in pss:
                    e = e_pool.tile([KT, QC], av_dt, tag="e")
                    if kt % NK in dve_grn_set:
                        st = sadd_pool.tile([KT, QC], fp32, tag="sadd")
                        nc.vector.tensor_add(
                            st[:, :qw], ps, g_all[:, kt, q0 : q0 + qw]
                        )
                        nc.scalar.activation(e[:, :qw], st[:, :qw], Exp)
                    else:
                        nc.scalar.activation(e[:, :qw], ps, Exp)
                    es.append(e)
                    pending.append((kt, e))
                if pt == 1 and pending_epilogue is not None:
                    # previous chunk's epilogue, traced here so its PE work
                    # (the bcast matmul) doesn't stall this chunk's k-loop
                    pending_epilogue()
                    pending_epilogue = None
            for i, e_prev in pending:
                nc.tensor.matmul(
                    po[:, :qw],
                    v_s[:, i, :],
                    e_prev[:, :qw],
                    start=(i == 0),
                    stop=(i == NK - 1),
                    skip_group_check=True,
                )
            pending_epilogue = make_epilogue(
                q0, qw, po, es, wg=2 if ci == len(chunks) - 1 else 4
            )
        pending_epilogue()

    nc.finalize()
    return nc


def _get_built():
    global _built
    if _built is None:
        _built = _build()
    return _built


def kernel(query, key, value, grn_bias, head_scale):
    global LAST_RESULT
    from concourse import bass_utils

    query = np.asarray(query, dtype=np.float32)
    key = np.asarray(key, dtype=np.float32)
    value = np.asarray(value, dtype=np.float32)
    grn_bias = np.asarray(grn_bias, dtype=np.float32)
    head_scale = np.asarray(head_scale, dtype=np.float32)

    import ml_dtypes

    grn_np = ml_dtypes.bfloat16 if BF16_GRN else np.float32
    qk_np = ml_dtypes.bfloat16 if BF16_QK else np.float32
    av_np = ml_dtypes.bfloat16 if BF16_AV else np.float32
    grnT_raw = grn_bias.T
    eye = np.eye(KT, dtype=grn_np)
    ones_col = np.ones((S, 1), dtype=np.float32)
    in_maps = []
    for h in range(N_CORES):
        qTh = query[h].T
        # pack K^T pairs: even k-tiles on rows 0-63, odd on 64-127
        kTp = np.ascontiguousarray(
            key[h].T.reshape(D, S // 256, 2, 128)
            .transpose(2, 0, 1, 3)
            .reshape(2 * D, S // 2)
        )
        in_maps.append(
            {
                "qT8": np.ascontiguousarray(
                    (np.vstack([qTh, qTh]) / np.float32(8.0)).astype(qk_np)
                ),
                "kT": kTp.astype(qk_np),
                "vaug": np.ascontiguousarray(
                    np.concatenate([value[h], ones_col], axis=1).astype(av_np)
                ),
                # grn^T pre-scaled by this head's scale -> the on-chip adds
                # (PE identity-matmul or DVE tensor_add) need no scaling
                "grnT": np.ascontiguousarray(
                    (head_scale[h] * grnT_raw).astype(grn_np)
                ),
                "hsI": eye,
            }
        )

    nc = _get_built()
    res = bass_utils.run_bass_kernel_spmd(
        nc, in_maps, core_ids=list(range(N_CORES)), trace=TRACE
    )
    LAST_RESULT = res

    attn = np.empty((1, H, S, S), dtype=np.float32)
    out = np.empty((1, H, S, D), dtype=np.float32)
    for h in range(N_CORES):
        attn[0, h] = res.results[h]["attnT"].T.astype(np.float32)
        out[0, h] = res.results[h]["outT"].T
    return out, attn


# revision 56
# speedup vs baseline: 1.2630x; 1.2630x over previous
"""GRN-biased attention (nn_GRNAttentionBias) Trainium2 Bass kernel.

Problem (hardcoded shapes): BH = H = 8 heads, S = 2048, D = 64.
  scores = Q @ K^T / sqrt(D) + head_scale[h] * grn_bias        (per head)
  attn   = softmax(scores, axis=-1)
  out    = attn @ V
  returns (out (1,H,S,D), attn (1,H,S,S))

Sharding: head-parallel — one head per NeuronCore (8 cores).

Kernel design (measured ~89 us/core on TRN2, vs ~99 us of raw HBM traffic
at per-core bandwidth for the fp32 version; bf16 streams bring the traffic
to ~19 MB/core):
  - Everything on-chip is computed TRANSPOSED (scores^T[k,q] instead of
    scores[q,k]) so the attn @ V contraction (over k) has k on partitions
    for both operands (V is naturally [k, d]; e^T is [k, q]) — no on-chip
    transpose of the 4.2M-element attention matrix is ever needed.
  - Row-sums of exp come free by augmenting V with a ones column (the AV
    matmul's 65th output row is Z).
  - grn_bias^T (host-transposed during sharding) is injected into the score
    PSUM by one extra accumulating matmul per tile: stationary = hs*I,
    moving = grn^T tile, computing += hs * grn^T.
  - softmax skips the max-subtraction: scores are bounded (|s| <~ 8 for
    randn Q,K with d=64 plus a [0,1] bias), exp stays in fp32 range.
  - 1/Z is broadcast across partitions on the otherwise-idle GPSIMD
    engine (partition_broadcast); normalize runs on DVE in the bf16 2x
    packed mode.
  - The q axis is processed in chunks [512,512,512,256,256] with each
    chunk's epilogue software-pipelined into the next chunk's k-loop; the
    narrower final chunks shorten the unavoidable post-Z tail.
  - grn^T stays SBUF-resident (bf16, 64KB/partition); its column loads are
    interleaved one chunk ahead so input DMA overlaps output bursts.
  - A few dummy matmuls at trace start warm the PE HAM clock gate while
    the first input DMAs stream in.
  - bf16 is used for the two dominant HBM streams (grn^T read, attn write)
    and the QK / AV matmul operands; PSUM accumulation and the final
    normalization stay fp32. Measured rel-l2 error vs the fp32 reference:
    ~4e-3 on both outputs (quantization-dominated, seed-stable).
The big outputs are written to HBM transposed ([k,q] / [d,q], fully
contiguous DMA) and transposed back on the host during unshard.

Host-side prep (part of sharding): per-head slices Q^T/8 (duplicated onto
both partition halves), K^T (even/odd k-tiles packed onto partition
halves), [V|1], hs*I, and one shared grn^T are computed in numpy.
"""

import os

import numpy as np

H = 8
S = 2048
D = 64
QC = 512  # q-chunk (PSUM bank = 512 fp32)
KT = 128  # k-tile (partition dim)
NQ = S // QC
NK = S // KT
N_CORES = 8

TRACE = bool(int(os.environ.get("KERNEL_TRACE", "0")))
# Halve the two dominant HBM streams (grn^T read, attn write) with bf16.
BF16_GRN = bool(int(os.environ.get("KERNEL_BF16_GRN", "1")))
BF16_ATTN = bool(int(os.environ.get("KERNEL_BF16_ATTN", "1")))
BF16_AV = bool(int(os.environ.get("KERNEL_BF16_AV", "1")))
BF16_QK = bool(int(os.environ.get("KERNEL_BF16_QK", "1")))
# k-tiles per chunk whose grn-bias add runs on DVE instead of a PE matmul
# (PE is the bottleneck engine; DVE has headroom)
DVE_GRN = int(os.environ.get("KERNEL_DVE_GRN", "0"))

_built = None
LAST_RESULT = None  # BassKernelResults from the most recent run (for test.py)


def _build():
    import concourse.mybir as mybir
    from contextlib import ExitStack
    from concourse import bacc
    from concourse.tile import TileContext

    fp32 = mybir.dt.float32
    Exp = mybir.ActivationFunctionType.Exp

    f32r = mybir.dt.float32r  # PE "rapid" fp32: 4x matmul throughput
    bf16 = mybir.dt.bfloat16
    grn_dt = bf16 if BF16_GRN else f32r
    attn_dt = bf16 if BF16_ATTN else fp32
    av_dt = bf16 if BF16_AV else f32r  # exp tiles + V for the AV matmul
    qk_dt = bf16 if BF16_QK else f32r  # Q^T/K^T operands (bf16 enables FWL)

    nc = bacc.Bacc("TRN2", target_bir_lowering=False)

    qT8 = nc.dram_tensor("qT8", [2 * D, S], qk_dt, kind="ExternalInput")
    kT = nc.dram_tensor("kT", [2 * D, S // 2], qk_dt, kind="ExternalInput")
    vaug = nc.dram_tensor("vaug", [S, D + 1], av_dt, kind="ExternalInput")
    grnT = nc.dram_tensor("grnT", [S, S], grn_dt, kind="ExternalInput")
    hsI = nc.dram_tensor("hsI", [KT, KT], grn_dt, kind="ExternalInput")
    attnT = nc.dram_tensor("attnT", [S, S], attn_dt, kind="ExternalOutput")
    outT = nc.dram_tensor("outT", [D, S], fp32, kind="ExternalOutput")

    with TileContext(nc) as tc, ExitStack() as ctx:
        singles = ctx.enter_context(tc.tile_pool(name="singles", bufs=1))
        grn_pool = ctx.enter_context(tc.tile_pool(name="grn", bufs=20))
        e_pool = ctx.enter_context(tc.tile_pool(name="e", bufs=40))
        a_pool = ctx.enter_context(tc.tile_pool(name="a", bufs=6))
        bc_pool = ctx.enter_context(tc.tile_pool(name="bc", bufs=2))
        sm_pool = ctx.enter_context(tc.tile_pool(name="sm", bufs=4))
        sadd_pool = ctx.enter_context(tc.tile_pool(name="sadd", bufs=3))
        ps_pool = ctx.enter_context(tc.tile_pool(name="ps", bufs=5, space="PSUM"))
        po_pool = ctx.enter_context(tc.tile_pool(name="po", bufs=2, space="PSUM"))
        pb_pool = ctx.enter_context(tc.tile_pool(name="pb", bufs=1, space="PSUM"))

        # chunks of the q axis: big chunks first, small last ones so the
        # unavoidable epilogue tail (normalize + attn writes after the final
        # Z is known) is short
        chunks = [(0, 512), (512, 512), (1024, 512), (1536, 256), (1792, 256)]
        # k-tiles whose grn add runs on DVE (spread across the loop)
        dve_grn_set = {3, 7, 11, 14, 1, 5, 9, 13}
        dve_grn_set = (
            set(sorted(dve_grn_set)[:DVE_GRN]) if (DVE_GRN and BF16_GRN) else set()
        )

        # startup-ordered input DMAs: everything chunk 0's k-loop touches
        # first, at fine granularity, so PE starts within a few us
        # row-packed for PE tile concurrency: qT_s = Q^T/8 duplicated onto
        # both partition halves; kT_s = K^T with even k-tiles on partitions
        # 0-63 and odd k-tiles on 64-127 (pair t occupies cols t*128..)
        qT_s = singles.tile([2 * D, S], qk_dt)
        kT_s = singles.tile([2 * D, S // 2], qk_dt)
        hsI_s = singles.tile([KT, KT], grn_dt)
        v_s = singles.tile([128, NK, D + 1], av_dt)  # V' = [V | 1]
        if BF16_GRN:
            g_all = singles.tile([128, NK, S], grn_dt, tag="g_all")
        else:
            g_all = None

        nc.sync.dma_start(out=hsI_s, in_=hsI[:, :])
        q0_0, qw_0 = chunks[0]
        nc.sync.dma_start(out=kT_s[:, 0:256], in_=kT[:, 0:256])
        nc.sync.dma_start(out=qT_s[:, q0_0 : qw_0], in_=qT8[:, q0_0 : qw_0])
        nc.sync.dma_start(out=kT_s[:, 256 : S // 2], in_=kT[:, 256 : S // 2])
        if g_all is not None:
            # first chunk's grn column in 4 sub-DMAs (k-tiles 0-3, 4-7, ...)
            for s in range(4):
                nc.sync.dma_start(
                    out=g_all[:, 4 * s : 4 * s + 4, q0_0 : qw_0],
                    in_=grnT[4 * s * KT : (4 * s + 4) * KT, q0_0 : qw_0].rearrange(
                        "(t p) q -> p t q", p=128
                    ),
                )
        nc.sync.dma_start(out=v_s, in_=vaug[:, :].rearrange("(t p) c -> p t c", p=128))
        for q0, qw in chunks[1:]:
            nc.sync.dma_start(out=qT_s[:, q0 : q0 + qw], in_=qT8[:, q0 : q0 + qw])
        ones1 = singles.tile([1, KT], fp32)
        nc.vector.memset(ones1, 1.0)
        # warm the PE (HAM clock gate) while the first input DMAs stream in:
        # dummy matmuls with no DMA dependencies (full-K shape: K=1 matmuls
        # are pathologically slow)
        warm_w = sm_pool.tile([KT, KT], fp32, tag="warm")
        nc.vector.memset(warm_w, 0.0)
        pwarm = pb_pool.tile([KT, QC], fp32, tag="pb")
        for _ in range(4):
            nc.tensor.matmul(
                pwarm[:, :KT], warm_w, warm_w, start=True, stop=True
            )

        def load_grn_col(q0, qw):
            if g_all is not None:
                nc.sync.dma_start(
                    out=g_all[:, :, q0 : q0 + qw],
                    in_=grnT[:, q0 : q0 + qw].rearrange("(t p) q -> p t q", p=128),
                )

        # chunk 1's grn column up front; later columns are queued one chunk
        # ahead inside the loop so input DMAs interleave with output bursts
        if len(chunks) > 1:
            load_grn_col(*chunks[1])

        def make_epilogue(q0, qw, po, es, wg=4):
            # everything that needs the finished po of chunk (q0, qw);
            # wg = k-tiles per attn write DMA (smaller on the final chunk so
            # the tail drains sooner)
            def epilogue():
                r = sm_pool.tile([1, QC], fp32, tag="r")
                nc.vector.reciprocal(r[:, :qw], po[D : D + 1, :qw])
                bc = bc_pool.tile([KT, QC], av_dt)
                # GPSIMD partition-broadcast: fast (~0.5us) and off the busy PE
                rb = sm_pool.tile([1, QC], av_dt, tag="rb")
                nc.vector.tensor_copy(rb[:, :qw], r[:, :qw])
                nc.gpsimd.partition_broadcast(bc[:, :qw], rb[:, :qw])

                for t in range(NK // wg):
                    a = a_pool.tile([KT, 4, QC], attn_dt, tag="a")
                    for j in range(wg):
                        nc.vector.tensor_mul(
                            a[:, j, :qw], es[wg * t + j][:, :qw], bc[:, :qw]
                        )
                    nc.sync.dma_start(
                        out=attnT[
                            wg * t * KT : (wg * t + wg) * KT, q0 : q0 + qw
                        ].rearrange("(j p) q -> p j q", p=128),
                        in_=a[:, :wg, :qw],
                    )

                ot = sm_pool.tile([D, QC], fp32, tag="ot")
                nc.vector.tensor_mul(ot[:, :qw], po[0:D, :qw], bc[0:D, :qw])
                nc.sync.dma_start(out=outT[:, q0 : q0 + qw], in_=ot[:, :qw])

            return epilogue

        pending_epilogue = None
        for ci, (q0, qw) in enumerate(chunks):
            if ci + 2 < len(chunks):
                load_grn_col(*chunks[ci + 2])
            po = po_pool.tile([D + 1, QC], fp32)
            es = []
            pending = []
            for pt in range(NK // 2):
                pss = []
                for b in range(2):
                    kt = 2 * pt + b
                    # scores^T[k, q] = sum_d K^T[d, k] * (Q^T/8)[d, q]
                    # even/odd k-tiles on partition halves -> the two QK
                    # matmuls run concurrently in the PE array (row groups)
                    ps_full = ps_pool.tile([KT, QC], fp32, tag="ps")
                    ps = ps_full[:, :qw]
                    on_dve = kt % NK in dve_grn_set
                    nc.tensor.matmul(
                        ps,
                        kT_s[b * D : (b + 1) * D, pt * KT : (pt + 1) * KT],
                        qT_s[b * D : (b + 1) * D, q0 : q0 + qw],
                        start=True,
                        stop=on_dve,
                        skip_group_check=True,
                    )
                    pss.append((kt, ps))
                for kt, ps in pss:
                    # += hs*grn^T: PE identity-matmul for most tiles; plain
                    # DVE add (into SBUF) for dve_grn_set to offload the
                    # bottleneck engine
                    if g_all is not None:
                        g = g_all[:, kt, q0 : q0 + qw]
                    else:
                        g_t = grn_pool.tile([KT, QC], grn_dt, tag="grn")
                        g = g_t[:, :qw]
                        nc.sync.dma_start(
                            out=g,
                            in_=grnT[kt * KT : (kt + 1) * KT, q0 : q0 + qw],
                        )
                    if kt % NK not in dve_grn_set:
                        nc.tensor.matmul(
                            ps, hsI_s, g, start=False, stop=True,
                            skip_group_check=True,
                        )
                # deferred AV matmuls so exp (ACT) overlaps PE
                for i, e_prev in pending:
                    nc.tensor.matmul(
                        po[:, :qw],
                        v_s[:, i, :],
                        e_prev[:, :qw],
                        start=(i == 0),
                        stop=False,
                        skip_group_check=True,
                    )
                pending = []
                for kt, ps in pss:
                    e = e_pool.tile([KT, QC], av_dt, tag="e")
                    if kt % NK in dve_grn_set:
                        st = sadd_pool.tile([KT, QC], fp32, tag="sadd")
                        nc.vector.tensor_add(
                            st[:, :qw], ps, g_all[:, kt, q0 : q0 + qw]
                        )
                        nc.scalar.activation(e[:, :qw], st[:, :qw], Exp)
                    else:
                        nc.scalar.activation(e[:, :qw], ps, Exp)
                    es.append(e)
                    pending.append((kt, e))
                if pt == 1 and pending_epilogue is not None:
                    # previous chunk's epilogue, traced here so its PE work
                    # (the bcast matmul) doesn't stall this chunk's k-loop
                    pending_epilogue()
                    pending_epilogue = None
            for i, e_prev in pending:
                nc.tensor.matmul(
                    po[:, :qw],
                    v_s[:, i, :],
                    e_prev[:, :qw],
                    start=(i == 0),
                    stop=(i == NK - 1),
                    skip_group_check=True,
                )
            pending_epilogue = make_epilogue(
                q0, qw, po, es, wg=2 if ci == len(chunks) - 1 else 4
            )
        pending_epilogue()

    nc.finalize()
    return nc


def _get_built():
    global _built
    if _built is None:
        _built = _build()
    return _built


def kernel(query, key, value, grn_bias, head_scale):
    global LAST_RESULT
    from concourse import bass_utils

    query = np.asarray(query, dtype=np.float32)
    key = np.asarray(key, dtype=np.float32)
    value = np.asarray(value, dtype=np.float32)
    grn_bias = np.asarray(grn_bias, dtype=np.float32)
    head_scale = np.asarray(head_scale, dtype=np.float32)

    import ml_dtypes

    grn_np = ml_dtypes.bfloat16 if BF16_GRN else np.float32
    qk_np = ml_dtypes.bfloat16 if BF16_QK else np.float32
    av_np = ml_dtypes.bfloat16 if BF16_AV else np.float32
    grnT_raw = grn_bias.T
    eye = np.eye(KT, dtype=grn_np)
    ones_col = np.ones((S, 1), dtype=np.float32)
    in_maps = []
    for h in range(N_CORES):
        qTh = query[h].T
        # pack K^T pairs: even k-tiles on rows 0-63, odd on 64-127
        kTp = np.ascontiguousarray(
            key[h].T.reshape(D, S // 256, 2, 128)
            .transpose(2, 0, 1, 3)
            .reshape(2 * D, S // 2)
        )
        in_maps.append(
            {
                "qT8": np.ascontiguousarray(
                    (np.vstack([qTh, qTh]) / np.float32(8.0)).astype(qk_np)
                ),
                "kT": kTp.astype(qk_np),
                "vaug": np.ascontiguousarray(
                    np.concatenate([value[h], ones_col], axis=1).astype(av_np)
                ),
                # grn^T pre-scaled by this head's scale -> the on-chip adds
                # (PE identity-matmul or DVE tensor_add) need no scaling
                "grnT": np.ascontiguousarray(
                    (head_scale[h] * grnT_raw).astype(grn_np)
                ),
                "hsI": eye,
            }
        )

    nc = _get_built()
    res = bass_utils.run_bass_kernel_spmd(
        nc, in_maps, core_ids=list(range(N_CORES)), trace=TRACE
    )
    LAST_RESULT = res

    attn = np.empty((1, H, S, S), dtype=np.float32)
    out = np.empty((1, H, S, D), dtype=np.float32)
    for h in range(N_CORES):
        attn[0, h] = res.results[h]["attnT"].T.astype(np.float32)
        out[0, h] = res.results[h]["outT"].T
    return out, attn
